# revision 1
# baseline (speedup 1.0000x reference)
"""MoE router gate kernel for Trainium2 (8 NeuronCores, SPMD data-parallel).

Reference computation (per problem nn_Gate_7241314861587):
    logits = x @ weight.T          # [8192, 4096] @ [4096, 256] -> [8192, 256]
    scores = sigmoid(logits)
    topv, indices = top_k(scores, 8)
    gates = topv / sum(topv)
    returns (gates f32 [8192, 8], indices int32 [8192, 8])

Strategy:
  - Data parallel: 1024 tokens per core; router weight replicated.
  - Host prepacks x and w into transposed (contraction-on-partition) fp16
    hi/lo splits.  logits = xh@wh + xh@wl + xl@wh accumulated in fp32 PSUM
    gives fp32-equivalent precision (~1e-6 abs err on logits; exact top-8
    indices) at fp16 matmul speed (3 cycles/row vs 4 for native fp32).
  - Weights stay SBUF-resident as [128, 32, 512] (wh ++ wl concat on the
    free axis) so the xh matmul covers both wh and wl halves in a single
    512-wide moving pass; xl@wh accumulates into the left half; one DVE
    add folds the halves.
  - Top-8 via the DVE MAX8 / FIND_INDEX_8 hardware (nc.vector.max /
    max_index): one instruction each per 128-token tile.
"""

import numpy as np

TOKENS, DIM, N_EXPERTS, TOPK = 8192, 4096, 256, 8
N_CORES = 8
TOK_SHARD = TOKENS // N_CORES     # 1024
TT = TOK_SHARD // 128             # 8 token tiles per core
KC = DIM // 128                   # 32 contraction chunks

_HALF = np.float16

_compiled = None


def _build():
    import concourse.mybir as mybir
    import concourse.tile as tile
    from concourse import bacc

    f32 = mybir.dt.float32
    f16 = mybir.dt.float16
    u32 = mybir.dt.uint32

    nc = bacc.Bacc("TRN2", target_bir_lowering=False, debug=False)

    xh_d = nc.dram_tensor("xh", [TT, 128, KC * 128], f16, kind="ExternalInput")
    xl_d = nc.dram_tensor("xl", [TT, 128, KC * 128], f16, kind="ExternalInput")
    w_d = nc.dram_tensor("wcat", [128, KC * 512], f16, kind="ExternalInput")
    gates_d = nc.dram_tensor("gates", [TOK_SHARD, TOPK], f32, kind="ExternalOutput")
    idx_d = nc.dram_tensor("idx", [TOK_SHARD, TOPK], u32, kind="ExternalOutput")

    with tile.TileContext(nc) as tc:
        with (
            tc.tile_pool(name="wp", bufs=1) as wp,
            tc.tile_pool(name="xp", bufs=4) as xp,
            tc.tile_pool(name="pp", bufs=4, space="PSUM") as pp,
            tc.tile_pool(name="sp", bufs=2) as sp,
        ):
            # Weight resident in SBUF; loaded in 8 chunks so the first
            # matmuls only wait on the first 512 KB, not the full 4 MB.
            wt = wp.tile([128, KC, 512], f16, tag="w")
            w_view = w_d[:].rearrange("p (kc e) -> p kc e", kc=KC)
            WCHUNK = 4
            for i, kc0 in enumerate(range(0, KC, WCHUNK)):
                eng = nc.sync if i % 2 == 0 else nc.scalar
                eng.dma_start(
                    wt[:, kc0:kc0 + WCHUNK, :], w_view[:, kc0:kc0 + WCHUNK, :]
                )

            for t in range(TT):
                xh_t = xp.tile([128, KC, 128], f16, tag="xh")
                xl_t = xp.tile([128, KC, 128], f16, tag="xl")
                XCHUNK = 8
                for kc0 in range(0, KC, XCHUNK):
                    nc.sync.dma_start(
                        xh_t[:, kc0:kc0 + XCHUNK, :],
                        xh_d[t].rearrange("p (kc n) -> p kc n", kc=KC)[
                            :, kc0:kc0 + XCHUNK, :
                        ],
                    )
                    nc.scalar.dma_start(
                        xl_t[:, kc0:kc0 + XCHUNK, :],
                        xl_d[t].rearrange("p (kc n) -> p kc n", kc=KC)[
                            :, kc0:kc0 + XCHUNK, :
                        ],
                    )

                # logits_hh ++ logits_hl accumulate in one 512-wide bank;
                # xl@wh folds into the left half.  One LDW per matmul, and
                # the xh pass covers both weight halves per instruction.
                ps = pp.tile([128, 512], f32, tag="ps")
                for k in range(KC):
                    if k > 0:
                        nc.tensor.matmul(
                            ps[:, 0:256], xl_t[:, k - 1, :], wt[:, k - 1, 0:256],
                            start=False, stop=False, skip_group_check=True,
                        )
                    nc.tensor.matmul(
                        ps[:], xh_t[:, k, :], wt[:, k, :],
                        start=(k == 0), stop=(k == KC - 1),
                        skip_group_check=True,
                    )
                nc.tensor.matmul(
                    ps[:, 0:256], xl_t[:, KC - 1, :], wt[:, KC - 1, 0:256],
                    start=False, stop=False, skip_group_check=True,
                )

                hl = sp.tile([128, 256], f32, tag="hl")
                nc.scalar.activation(
                    hl[:], ps[:, 256:512], mybir.ActivationFunctionType.Copy
                )
                pre = sp.tile([128, 256], f32, tag="pre")
                nc.vector.tensor_add(pre[:], ps[:, 0:256], hl[:])
                scores = sp.tile([128, 256], f32, tag="scores")
                nc.scalar.activation(
                    scores[:], pre[:], mybir.ActivationFunctionType.Sigmoid
                )

                top = sp.tile([128, TOPK], f32, tag="top")
                idxt = sp.tile([128, TOPK], u32, tag="idxt")
                nc.vector.max(out=top[:], in_=scores[:])
                nc.vector.max_index(out=idxt[:], in_max=top[:], in_values=scores[:])

                ssum = sp.tile([128, 1], f32, tag="ssum")
                nc.vector.reduce_sum(ssum[:], top[:], axis=mybir.AxisListType.X)
                rec = sp.tile([128, 1], f32, tag="rec")
                nc.vector.reciprocal(rec[:], ssum[:])
                gt = sp.tile([128, TOPK], f32, tag="gt")
                nc.vector.tensor_scalar_mul(gt[:], top[:], rec[:])

                nc.sync.dma_start(gates_d[t * 128:(t + 1) * 128, :], gt[:])
                nc.sync.dma_start(idx_d[t * 128:(t + 1) * 128, :], idxt[:])

    nc.compile()
    return nc


def _prep_inputs(x, weight):
    """Host-side shard + transpose + fp16 hi/lo split -> per-core in_maps."""
    x = np.ascontiguousarray(np.asarray(x, dtype=np.float32))
    w = np.ascontiguousarray(np.asarray(weight, dtype=np.float32))

    # Weight: wcat[p, kc*512 + e'] with e' = [wh(256) ++ wl(256)]
    wT = np.ascontiguousarray(w.T)                     # [4096, 256]
    wh = wT.astype(_HALF)
    wl = (wT - wh.astype(np.float32)).astype(_HALF)
    wcat = np.concatenate([wh, wl], axis=1)            # [4096, 512]
    wcat = wcat.reshape(KC, 128, 512).transpose(1, 0, 2).reshape(128, KC * 512)
    wcat = np.ascontiguousarray(wcat)

    xh = x.astype(_HALF)
    xl = (x - xh.astype(np.float32)).astype(_HALF)

    in_maps = []
    for c in range(N_CORES):
        sl = slice(c * TOK_SHARD, (c + 1) * TOK_SHARD)
        maps = {}
        for name, arr in (("xh", xh[sl]), ("xl", xl[sl])):
            # [1024, 4096] -> [t, tok, kc, p] -> [t, p, kc, tok]
            a = arr.reshape(TT, 128, KC, 128).transpose(0, 3, 2, 1)
            maps[name] = np.ascontiguousarray(a.reshape(TT, 128, KC * 128))
        maps["wcat"] = wcat
        in_maps.append(maps)
    return in_maps


def kernel(x, weight, _trace=False, _trace_kwargs=None):
    global _compiled
    from concourse.bass_utils import run_bass_kernel_spmd

    if _compiled is None:
        _compiled = _build()

    in_maps = _prep_inputs(x, weight)
    res = run_bass_kernel_spmd(
        _compiled,
        in_maps,
        core_ids=list(range(N_CORES)),
        trace=_trace,
        **(_trace_kwargs or {}),
    )

    gates = np.concatenate([r["gates"] for r in res.results], axis=0)
    idx = np.concatenate(
        [r["idx"].astype(np.int32) for r in res.results], axis=0
    )
    if _trace:
        kernel.last_results = res
    return gates, idx



# revision 3
# speedup vs baseline: 1.0544x; 1.0544x over previous
"""MoE router gate kernel for Trainium2 (8 NeuronCores, SPMD data-parallel).

Reference computation (per problem nn_Gate_7241314861587):
    logits = x @ weight.T          # [8192, 4096] @ [4096, 256] -> [8192, 256]
    scores = sigmoid(logits)
    topv, indices = top_k(scores, 8)
    gates = topv / sum(topv)
    returns (gates f32 [8192, 8], indices int32 [8192, 8])

Strategy (v2):
  - Data parallel: 1024 tokens per core; router weight replicated.
  - logits = xh@wh (fp16 main) + xl@wh (fp8 DoubleRow) + x@wl (fp8 DoubleRow)
    where xh = fp16(x), xl = e4m3((x - xh) * 2^15) DMA'd from host,
    wh = fp16(w), wl8 = e4m3((w - wh) * 2^18) DMA'd from host,
    wh8 = e4m3(wh * 2^8) and x08 = e4m3(xh) cast on-chip (ACT/DVE).
    Logit error ~1e-5 -> a couple of top-8 boundary swaps out of 65536.
  - fp16 main: 32 matmuls/tile at 256 moving cols (1 cyc/row).
    Corrections: 16 DoubleRow matmuls each (K=256/chunk, 0.5 cyc/row).
  - DMA per core 15 MB (vs 20 MB baseline); PE ~41us (vs 82us baseline).
  - Top-8 via DVE MAX8 / FIND_INDEX_8 as baseline.
"""

import numpy as np

TOKENS, DIM, N_EXPERTS, TOPK = 8192, 4096, 256, 8
N_CORES = 8
TOK_SHARD = TOKENS // N_CORES     # 1024
TT = TOK_SHARD // 128             # 8 token tiles per core
KC = DIM // 128                   # 32 contraction chunks (fp16 pass)
KC2 = DIM // 256                  # 16 contraction chunks (DoubleRow pass)

XL_S = 2.0 ** 15                  # xl8 holds (x - xh) * XL_S
WL_S = 2.0 ** 18                  # wl8 holds (w - wh) * WL_S
WH8_S = 2.0 ** 8                  # wh8 holds wh * WH8_S

_compiled = None


def _build():
    import concourse.mybir as mybir
    import concourse.tile as tile
    from concourse import bacc

    f32 = mybir.dt.float32
    f16 = mybir.dt.float16
    f8 = mybir.dt.float8e4
    u32 = mybir.dt.uint32
    DR = mybir.MatmulPerfMode.DoubleRow

    nc = bacc.Bacc("TRN2", target_bir_lowering=False, debug=False)

    xh_d = nc.dram_tensor("xh", [TT, 128, KC * 128], f16, kind="ExternalInput")
    xl_d = nc.dram_tensor("xl8", [TT, 128, KC2 * 2 * 128], f8, kind="ExternalInput")
    wh_d = nc.dram_tensor("wh", [128, KC * 256], f16, kind="ExternalInput")
    wl_d = nc.dram_tensor("wl8", [128, KC2 * 2 * 256], f8, kind="ExternalInput")
    gates_d = nc.dram_tensor("gates", [TOK_SHARD, TOPK], f32, kind="ExternalOutput")
    idx_d = nc.dram_tensor("idx", [TOK_SHARD, TOPK], u32, kind="ExternalOutput")

    with tile.TileContext(nc) as tc:
        with (
            tc.tile_pool(name="wp", bufs=1) as wp,
            tc.tile_pool(name="xp", bufs=4) as xp,
            tc.tile_pool(name="pp", bufs=2, space="PSUM") as pp,
            tc.tile_pool(name="sp", bufs=2) as sp,
        ):
            # --- weights resident in SBUF ---
            # DMA order matters: first wh chunk + first x tile gate the first
            # matmul, so they go first; the rest streams behind.
            wht = wp.tile([128, KC, 256], f16, tag="wh")
            wh_v = wh_d[:].rearrange("p (kc e) -> p kc e", kc=KC)
            WCHUNK = 8
            nc.sync.dma_start(wht[:, 0:WCHUNK, :], wh_v[:, 0:WCHUNK, :])

            xh_tiles = []
            xl_tiles = []
            x08_tiles = []
            XCHUNK = 8

            def load_xh(t, split=1):
                xh_t = xp.tile([128, KC, 128], f16, tag="xh")
                step = KC // split
                for kc0 in range(0, KC, step):
                    nc.sync.dma_start(
                        xh_t[:, kc0:kc0 + step, :],
                        xh_d[t].rearrange("p (kc n) -> p kc n", kc=KC)[
                            :, kc0:kc0 + step, :
                        ],
                    )
                xh_tiles.append(xh_t)

            def load_xl(t):
                xl_t = xp.tile([128, KC2, 2, 128], f8, tag="xl")
                nc.gpsimd.dma_start(
                    xl_t[:],
                    xl_d[t].rearrange("p (k two n) -> p k two n", k=KC2, two=2),
                )
                xl_tiles.append(xl_t)

            def cast_x08(t):
                # x08 = e4m3(xh): rotate the big cast across ACT/DVE/Pool so
                # no single engine becomes the bottleneck
                xh_t = xh_tiles[t]
                x08_t = xp.tile([128, KC, 128], f8, tag="x08")
                if t in (2, 5, 7):
                    nc.vector.tensor_copy(x08_t[:], xh_t[:])
                else:
                    nc.scalar.activation(
                        x08_t[:], xh_t[:], mybir.ActivationFunctionType.Copy
                    )
                x08_tiles.append(x08_t)

            load_xh(0, split=4)
            for kc0 in range(WCHUNK, KC, WCHUNK):
                nc.sync.dma_start(
                    wht[:, kc0:kc0 + WCHUNK, :], wh_v[:, kc0:kc0 + WCHUNK, :]
                )
            load_xl(0)
            wlt = wp.tile([128, KC2, 2, 256], f8, tag="wl")
            nc.sync.dma_start(
                wlt[:], wl_d[:].rearrange("p (k two e) -> p k two e", k=KC2, two=2)
            )
            # wh8 = e4m3(wh * 2^8), cast on-chip (ACT), viewed as DR layout
            wh8t = wp.tile([128, KC, 256], f8, tag="wh8")
            for kc0 in range(0, KC, 16):
                nc.scalar.activation(
                    wh8t[:, kc0:kc0 + 16, :], wht[:, kc0:kc0 + 16, :],
                    mybir.ActivationFunctionType.Copy, scale=WH8_S,
                )
            wh8_dr = wh8t[:].rearrange("p (k two) e -> p k two e", two=2)
            cast_x08(0)
            load_xh(1)
            load_xl(1)
            cast_x08(1)
            gsb = wp.tile([128, TT, TOPK], f32, tag="gsb")
            isb = wp.tile([128, TT, TOPK], u32, tag="isb")

            for t in range(TT):
                if t + 2 < TT:
                    load_xh(t + 2, split=2)
                    load_xl(t + 2)
                xh_t = xh_tiles[t]
                xl_t = xl_tiles[t]
                x08_dr = x08_tiles[t][:].rearrange(
                    "p (k two) n -> p k two n", two=2
                )

                ps1 = pp.tile([128, 256], f32, tag="ps1")
                ps3 = pp.tile([128, 256], f32, tag="ps3")
                ps2 = pp.tile([128, 256], f32, tag="ps2")
                for k in range(KC):
                    nc.tensor.matmul(
                        ps1[:], xh_t[:, k, :], wht[:, k, :],
                        start=(k == 0), stop=(k == KC - 1),
                        skip_group_check=True,
                    )
                for k in range(KC2):
                    nc.tensor.matmul(
                        ps3[:], xl_t[:, k, :, :], wh8_dr[:, k, :, :],
                        start=(k == 0), stop=(k == KC2 - 1),
                        perf_mode=DR, skip_group_check=True,
                    )
                for k in range(KC2):
                    nc.tensor.matmul(
                        ps2[:], x08_dr[:, k, :, :], wlt[:, k, :, :],
                        start=(k == 0), stop=(k == KC2 - 1),
                        perf_mode=DR, skip_group_check=True,
                    )

                # fold: pre = ps1 + ps3/(XL_S*WH8_S) + ps2/WL_S
                # (a DVE op may read at most one PSUM operand -> stage ps3
                #  through an ACT copy that also applies its descale)
                t3s = sp.tile([128, 256], f32, tag="t3s")
                nc.scalar.activation(
                    t3s[:], ps3[:], mybir.ActivationFunctionType.Copy,
                    scale=1.0 / (XL_S * WH8_S),
                )
                tmp = sp.tile([128, 256], f32, tag="tmp")
                nc.vector.scalar_tensor_tensor(
                    tmp[:], ps1[:], 1.0, t3s[:],
                    op0=mybir.AluOpType.mult, op1=mybir.AluOpType.add,
                )
                pre = sp.tile([128, 256], f32, tag="pre")
                nc.vector.scalar_tensor_tensor(
                    pre[:], ps2[:], 1.0 / WL_S, tmp[:],
                    op0=mybir.AluOpType.mult, op1=mybir.AluOpType.add,
                )
                scores = sp.tile([128, 256], f32, tag="scores")
                nc.scalar.activation(
                    scores[:], pre[:], mybir.ActivationFunctionType.Sigmoid
                )

                top = sp.tile([128, TOPK], f32, tag="top")
                idxt = sp.tile([128, TOPK], u32, tag="idxt")
                nc.vector.max(out=top[:], in_=scores[:])
                nc.vector.max_index(out=idxt[:], in_max=top[:], in_values=scores[:])

                ssum = sp.tile([128, 1], f32, tag="ssum")
                nc.vector.reduce_sum(ssum[:], top[:], axis=mybir.AxisListType.X)
                rec = sp.tile([128, 1], f32, tag="rec")
                nc.vector.reciprocal(rec[:], ssum[:])
                nc.vector.tensor_scalar_mul(gsb[:, t, :], top[:], rec[:])
                nc.vector.tensor_copy(isb[:, t, :], idxt[:])
                if t + 2 < TT:
                    cast_x08(t + 2)

            nc.sync.dma_start(
                gates_d[:].rearrange("(t p) k -> p t k", t=TT), gsb[:]
            )
            nc.sync.dma_start(
                idx_d[:].rearrange("(t p) k -> p t k", t=TT), isb[:]
            )

    nc.compile()
    return nc


def _prep_inputs(x, weight):
    """Host-side shard + transpose + fp16/fp8 split -> per-core in_maps."""
    from concourse.dt import dt as cdt
    import concourse.mybir as mybir

    F8 = cdt.np(mybir.dt.float8e4)

    x = np.ascontiguousarray(np.asarray(x, dtype=np.float32))
    w = np.ascontiguousarray(np.asarray(weight, dtype=np.float32))

    wT = np.ascontiguousarray(w.T)                     # [4096, 256]
    whT = wT.astype(np.float16)
    wlT = ((wT - whT.astype(np.float32)) * WL_S).astype(F8)
    wh_m = np.ascontiguousarray(
        whT.reshape(KC, 128, 256).transpose(1, 0, 2).reshape(128, KC * 256)
    )
    wl_m = np.ascontiguousarray(
        wlT.reshape(KC2, 2, 128, 256).transpose(2, 0, 1, 3).reshape(128, KC2 * 2 * 256)
    )

    xh = x.astype(np.float16)
    xl8 = ((x - xh.astype(np.float32)) * XL_S).astype(F8)

    in_maps = []
    for c in range(N_CORES):
        sl = slice(c * TOK_SHARD, (c + 1) * TOK_SHARD)
        # xh: [1024, 4096] -> [t, tok, kc, dp] -> [t, dp, kc, tok]
        a = xh[sl].reshape(TT, 128, KC, 128).transpose(0, 3, 2, 1)
        # xl8: [t, tok, k2, i, dp] -> [t, dp, k2, i, tok]
        b = xl8[sl].reshape(TT, 128, KC2, 2, 128).transpose(0, 4, 2, 3, 1)
        in_maps.append({
            "xh": np.ascontiguousarray(a.reshape(TT, 128, KC * 128)),
            "xl8": np.ascontiguousarray(b.reshape(TT, 128, KC2 * 2 * 128)),
            "wh": wh_m,
            "wl8": wl_m,
        })
    return in_maps


def kernel(x, weight, _trace=False, _trace_kwargs=None):
    global _compiled
    from concourse.bass_utils import run_bass_kernel_spmd

    if _compiled is None:
        _compiled = _build()

    in_maps = _prep_inputs(x, weight)
    res = run_bass_kernel_spmd(
        _compiled,
        in_maps,
        core_ids=list(range(N_CORES)),
        trace=_trace,
        **(_trace_kwargs or {}),
    )

    gates = np.concatenate([r["gates"] for r in res.results], axis=0)
    idx = np.concatenate(
        [r["idx"].astype(np.int32) for r in res.results], axis=0
    )
    if _trace:
        kernel.last_results = res
    return gates, idx


# revision 4
# speedup vs baseline: 1.0734x; 1.0180x over previous
"""MoE router gate kernel for Trainium2 (8 NeuronCores, SPMD data-parallel).

Reference computation (per problem nn_Gate_7241314861587):
    logits = x @ weight.T          # [8192, 4096] @ [4096, 256] -> [8192, 256]
    scores = sigmoid(logits)
    topv, indices = top_k(scores, 8)
    gates = topv / sum(topv)
    returns (gates f32 [8192, 8], indices int32 [8192, 8])

Strategy (v2):
  - Data parallel: 1024 tokens per core; router weight replicated.
  - logits = xh@wh (fp16 main) + xl@wh (fp8 DoubleRow) + x@wl (fp8 DoubleRow)
    where xh = fp16(x), xl = e4m3((x - xh) * 2^15) DMA'd from host,
    wh = fp16(w), wl8 = e4m3((w - wh) * 2^18) DMA'd from host,
    wh8 = e4m3(wh * 2^8) and x08 = e4m3(xh) cast on-chip (ACT/DVE).
    Logit error ~1e-5 -> a couple of top-8 boundary swaps out of 65536.
  - fp16 main: 32 matmuls/tile at 256 moving cols (1 cyc/row).
    Corrections: 16 DoubleRow matmuls each (K=256/chunk, 0.5 cyc/row).
  - DMA per core 15 MB (vs 20 MB baseline); PE ~41us (vs 82us baseline).
  - Top-8 via DVE MAX8 / FIND_INDEX_8 as baseline.
"""

import numpy as np

TOKENS, DIM, N_EXPERTS, TOPK = 8192, 4096, 256, 8
N_CORES = 8
TOK_SHARD = TOKENS // N_CORES     # 1024
TT = TOK_SHARD // 128             # 8 token tiles per core
KC = DIM // 128                   # 32 contraction chunks (fp16 pass)
KC2 = DIM // 256                  # 16 contraction chunks (DoubleRow pass)

XL_S = 2.0 ** 15                  # xl8 holds (x - xh) * XL_S
WL_S = 2.0 ** 18                  # wl8 holds (w - wh) * WL_S
WH8_S = 2.0 ** 8                  # wh8 holds wh * WH8_S

_compiled = None


def _build():
    import concourse.mybir as mybir
    import concourse.tile as tile
    from concourse import bacc

    f32 = mybir.dt.float32
    f16 = mybir.dt.float16
    f8 = mybir.dt.float8e4
    u32 = mybir.dt.uint32
    DR = mybir.MatmulPerfMode.DoubleRow

    nc = bacc.Bacc("TRN2", target_bir_lowering=False, debug=False)

    xh_d = nc.dram_tensor("xh", [TT, 128, KC * 128], f16, kind="ExternalInput")
    xl_d = nc.dram_tensor("xl8", [TT, 128, KC2 * 2 * 128], f8, kind="ExternalInput")
    wh_d = nc.dram_tensor("wh", [128, KC * 256], f16, kind="ExternalInput")
    wl_d = nc.dram_tensor("wl8", [128, KC2 * 2 * 256], f8, kind="ExternalInput")
    gates_d = nc.dram_tensor("gates", [TOK_SHARD, TOPK], f32, kind="ExternalOutput")
    idx_d = nc.dram_tensor("idx", [TOK_SHARD, TOPK], u32, kind="ExternalOutput")

    with tile.TileContext(nc) as tc:
        with (
            tc.tile_pool(name="wp", bufs=1) as wp,
            tc.tile_pool(name="xp", bufs=4) as xp,
            tc.tile_pool(name="pp", bufs=2, space="PSUM") as pp,
            tc.tile_pool(name="sp", bufs=2) as sp,
        ):
            # --- weights resident in SBUF ---
            # DMA order matters: first wh chunk + first x tile gate the first
            # matmul, so they go first; the rest streams behind.
            wht = wp.tile([128, KC, 256], f16, tag="wh")
            wh_v = wh_d[:].rearrange("p (kc e) -> p kc e", kc=KC)
            WCHUNK = 8
            nc.sync.dma_start(wht[:, 0:WCHUNK, :], wh_v[:, 0:WCHUNK, :])

            xh_tiles = []
            xl_tiles = []
            x08_tiles = []
            XCHUNK = 8

            def load_xh(t, split=1):
                xh_t = xp.tile([128, KC, 128], f16, tag="xh")
                step = KC // split
                for kc0 in range(0, KC, step):
                    nc.sync.dma_start(
                        xh_t[:, kc0:kc0 + step, :],
                        xh_d[t].rearrange("p (kc n) -> p kc n", kc=KC)[
                            :, kc0:kc0 + step, :
                        ],
                    )
                xh_tiles.append(xh_t)

            def load_xl(t):
                xl_t = xp.tile([128, KC2, 2, 128], f8, tag="xl")
                nc.gpsimd.dma_start(
                    xl_t[:],
                    xl_d[t].rearrange("p (k two n) -> p k two n", k=KC2, two=2),
                )
                xl_tiles.append(xl_t)

            def cast_x08(t):
                # x08 = e4m3(xh): rotate the big cast across ACT/DVE/Pool so
                # no single engine becomes the bottleneck
                xh_t = xh_tiles[t]
                x08_t = xp.tile([128, KC, 128], f8, tag="x08")
                if t in (2, 5, 7):
                    nc.vector.tensor_copy(x08_t[:], xh_t[:])
                else:
                    nc.scalar.activation(
                        x08_t[:], xh_t[:], mybir.ActivationFunctionType.Copy
                    )
                x08_tiles.append(x08_t)

            load_xh(0, split=4)
            for kc0 in range(WCHUNK, KC, WCHUNK):
                nc.sync.dma_start(
                    wht[:, kc0:kc0 + WCHUNK, :], wh_v[:, kc0:kc0 + WCHUNK, :]
                )
            load_xl(0)
            wlt = wp.tile([128, KC2, 2, 256], f8, tag="wl")
            wl_v = wl_d[:].rearrange("p (k two e) -> p k two e", k=KC2, two=2)
            nc.sync.dma_start(wlt[:, 0:8, :, :], wl_v[:, 0:8, :, :])
            nc.sync.dma_start(wlt[:, 8:KC2, :, :], wl_v[:, 8:KC2, :, :])
            # wh8 = e4m3(wh * 2^8), cast on-chip (ACT), viewed as DR layout
            wh8t = wp.tile([128, KC, 256], f8, tag="wh8")
            for kc0 in range(0, KC, 16):
                nc.scalar.activation(
                    wh8t[:, kc0:kc0 + 16, :], wht[:, kc0:kc0 + 16, :],
                    mybir.ActivationFunctionType.Copy, scale=WH8_S,
                )
            wh8_dr = wh8t[:].rearrange("p (k two) e -> p k two e", two=2)
            cast_x08(0)
            load_xh(1)
            load_xl(1)
            cast_x08(1)
            gsb = wp.tile([128, TT, TOPK], f32, tag="gsb")
            isb = wp.tile([128, TT, TOPK], u32, tag="isb")

            for t in range(TT):
                if t + 2 < TT:
                    load_xh(t + 2, split=2)
                    load_xl(t + 2)
                xh_t = xh_tiles[t]
                xl_t = xl_tiles[t]
                x08_dr = x08_tiles[t][:].rearrange(
                    "p (k two) n -> p k two n", two=2
                )

                ps1 = pp.tile([128, 256], f32, tag="ps1")
                ps3 = pp.tile([128, 256], f32, tag="ps3")
                ps2 = pp.tile([128, 256], f32, tag="ps2")
                for k in range(KC):
                    nc.tensor.matmul(
                        ps1[:], xh_t[:, k, :], wht[:, k, :],
                        start=(k == 0), stop=(k == KC - 1),
                        skip_group_check=True,
                    )
                for k in range(KC2):
                    nc.tensor.matmul(
                        ps3[:], xl_t[:, k, :, :], wh8_dr[:, k, :, :],
                        start=(k == 0), stop=(k == KC2 - 1),
                        perf_mode=DR, skip_group_check=True,
                    )
                for k in range(KC2):
                    nc.tensor.matmul(
                        ps2[:], x08_dr[:, k, :, :], wlt[:, k, :, :],
                        start=(k == 0), stop=(k == KC2 - 1),
                        perf_mode=DR, skip_group_check=True,
                    )

                # fold: pre = ps1 + ps3/(XL_S*WH8_S) + ps2/WL_S
                # (a DVE op may read at most one PSUM operand -> stage ps3
                #  through an ACT copy that also applies its descale)
                t3s = sp.tile([128, 256], f32, tag="t3s")
                nc.scalar.activation(
                    t3s[:], ps3[:], mybir.ActivationFunctionType.Copy,
                    scale=1.0 / (XL_S * WH8_S),
                )
                tmp = sp.tile([128, 256], f32, tag="tmp")
                nc.vector.scalar_tensor_tensor(
                    tmp[:], ps1[:], 1.0, t3s[:],
                    op0=mybir.AluOpType.mult, op1=mybir.AluOpType.add,
                )
                pre = sp.tile([128, 256], f32, tag="pre")
                nc.vector.scalar_tensor_tensor(
                    pre[:], ps2[:], 1.0 / WL_S, tmp[:],
                    op0=mybir.AluOpType.mult, op1=mybir.AluOpType.add,
                )
                scores = sp.tile([128, 256], f32, tag="scores")
                nc.scalar.activation(
                    scores[:], pre[:], mybir.ActivationFunctionType.Sigmoid
                )

                top = sp.tile([128, TOPK], f32, tag="top")
                idxt = sp.tile([128, TOPK], u32, tag="idxt")
                nc.vector.max(out=top[:], in_=scores[:])
                nc.vector.max_index(out=idxt[:], in_max=top[:], in_values=scores[:])

                ssum = sp.tile([128, 1], f32, tag="ssum")
                nc.vector.reduce_sum(ssum[:], top[:], axis=mybir.AxisListType.X)
                rec = sp.tile([128, 1], f32, tag="rec")
                nc.vector.reciprocal(rec[:], ssum[:])
                nc.vector.tensor_scalar_mul(gsb[:, t, :], top[:], rec[:])
                nc.vector.tensor_copy(isb[:, t, :], idxt[:])
                if t + 2 < TT:
                    cast_x08(t + 2)

            nc.sync.dma_start(
                gates_d[:].rearrange("(t p) k -> p t k", t=TT), gsb[:]
            )
            nc.sync.dma_start(
                idx_d[:].rearrange("(t p) k -> p t k", t=TT), isb[:]
            )

    nc.compile()
    return nc


def _prep_inputs(x, weight):
    """Host-side shard + transpose + fp16/fp8 split -> per-core in_maps."""
    from concourse.dt import dt as cdt
    import concourse.mybir as mybir

    F8 = cdt.np(mybir.dt.float8e4)

    x = np.ascontiguousarray(np.asarray(x, dtype=np.float32))
    w = np.ascontiguousarray(np.asarray(weight, dtype=np.float32))

    wT = np.ascontiguousarray(w.T)                     # [4096, 256]
    whT = wT.astype(np.float16)
    wlT = ((wT - whT.astype(np.float32)) * WL_S).astype(F8)
    wh_m = np.ascontiguousarray(
        whT.reshape(KC, 128, 256).transpose(1, 0, 2).reshape(128, KC * 256)
    )
    wl_m = np.ascontiguousarray(
        wlT.reshape(KC2, 2, 128, 256).transpose(2, 0, 1, 3).reshape(128, KC2 * 2 * 256)
    )

    xh = x.astype(np.float16)
    xl8 = ((x - xh.astype(np.float32)) * XL_S).astype(F8)

    in_maps = []
    for c in range(N_CORES):
        sl = slice(c * TOK_SHARD, (c + 1) * TOK_SHARD)
        # xh: [1024, 4096] -> [t, tok, kc, dp] -> [t, dp, kc, tok]
        a = xh[sl].reshape(TT, 128, KC, 128).transpose(0, 3, 2, 1)
        # xl8: [t, tok, k2, i, dp] -> [t, dp, k2, i, tok]
        b = xl8[sl].reshape(TT, 128, KC2, 2, 128).transpose(0, 4, 2, 3, 1)
        in_maps.append({
            "xh": np.ascontiguousarray(a.reshape(TT, 128, KC * 128)),
            "xl8": np.ascontiguousarray(b.reshape(TT, 128, KC2 * 2 * 128)),
            "wh": wh_m,
            "wl8": wl_m,
        })
    return in_maps


def kernel(x, weight, _trace=False, _trace_kwargs=None):
    global _compiled
    from concourse.bass_utils import run_bass_kernel_spmd

    if _compiled is None:
        _compiled = _build()

    in_maps = _prep_inputs(x, weight)
    res = run_bass_kernel_spmd(
        _compiled,
        in_maps,
        core_ids=list(range(N_CORES)),
        trace=_trace,
        **(_trace_kwargs or {}),
    )

    gates = np.concatenate([r["gates"] for r in res.results], axis=0)
    idx = np.concatenate(
        [r["idx"].astype(np.int32) for r in res.results], axis=0
    )
    if _trace:
        kernel.last_results = res
    return gates, idx
